# revision 20
# baseline (speedup 1.0000x reference)
"""Two-layer GCN (PyG GCNConv x2 with tanh) on 8 Trainium2 NeuronCores.

Strategy (graph/data parallel, per sharding hint):
  - Nodes are dealt round-robin by degree rank across the 8 cores (balances
    edge counts), then degree-sorted within each core into blocks of 128.
  - The "table" (per-layer per-node feature rows) lives in DRAM in permuted
    node order: 8 chunks of 6252 rows (2 leading zero rows per chunk, used
    as gather padding targets), AllGathered across cores after each local
    transform.
  - Normalization trick: out[d] = dinv[d] * sum_{e:dst=d} (dinv[s]*xw[s]),
    so the table rows are pre-scaled by dinv and the aggregation is a pure
    unweighted segment-sum of gathered rows.
  - Aggregation: per 128-node block, dma_gather pulls all incident edges'
    table rows into SBUF slots [partition=node, column=edge j], and the DVE
    tree-reduces columns. Gather indices are int16, so each gather call uses
    one of two overlapping table windows (rows [0,32768) / [18756,50016))
    with per-edge window assignment balanced at preprocessing time.
  - Layer 1 transforms then aggregates (128-dim rows); layer 2 transforms
    (h @ W2) then aggregates 64-dim rows.
"""
import sys

if "/opt/trn_rl_repo" not in sys.path:
    sys.path.insert(0, "/opt/trn_rl_repo")

import numpy as np

import concourse.bacc as bacc
import concourse.mybir as mybir
import concourse.tile as tile
from concourse.bass_interp import MultiCoreSim

# -------- problem constants (hardcoded; kernel.py must be self-contained) ----
N_NODES = 50000
IN_DIM, HID_DIM, OUT_DIM = 256, 128, 64
N_CORES = 8
PER_CORE = N_NODES // N_CORES          # 6250
SLICE = PER_CORE + 2                   # 6252 (2 zero rows per core chunk)
TBL = SLICE * N_CORES                  # 50016
WIN_A_LEN = 32768                      # window A: rows [0, 32768)
WIN_B_BASE = 3 * SLICE                 # 18756 == a zero row (core 3, row 0)
WIN_B_LEN = TBL - WIN_B_BASE           # 31260 (max idx 31259 < 32768)
BLK = 128
NBLK = (PER_CORE + BLK - 1) // BLK     # 49 (last block has 106 real nodes)
GROUP = 2                              # blocks per gather call group
F32 = mybir.dt.float32
I16 = mybir.dt.int16


def _wrap_idxs(idx_flat: np.ndarray) -> np.ndarray:
    """Unwrapped idx stream [n] -> [128, n//16] int16 (16-partition wrap,
    replicated 8x for the 8 Q7 cores)."""
    n = idx_flat.shape[0]
    assert n % 16 == 0
    base = idx_flat.reshape(n // 16, 16).T.astype(np.int16)
    return np.tile(base, (8, 1))


def _preprocess(edge_index: np.ndarray):
    """Host-side integer/index preprocessing. Returns shared structure
    (block grid dims, group layout) and per-core staged arrays."""
    deg = np.bincount(edge_index[1].astype(np.int64), minlength=N_NODES) + 1
    deg = deg.astype(np.int64)  # in-degree including self loop

    # node -> (core, pos, table row): round-robin deal in degree-desc order
    order = np.argsort(-deg, kind="stable")          # rank -> node
    rank = np.empty(N_NODES, np.int64)
    rank[order] = np.arange(N_NODES)
    core_of = rank % N_CORES
    pos_of = rank // N_CORES                          # 0..6249 within core
    row_of = core_of * SLICE + 2 + pos_of             # table row of node
    # processing key: core*PER_CORE + pos (0..49999)
    pkey_of = core_of * PER_CORE + pos_of

    # edges incl self loops, as (src_row, dst_pkey)
    src = np.concatenate([edge_index[0].astype(np.int64),
                          np.arange(N_NODES, dtype=np.int64)])
    dst = np.concatenate([edge_index[1].astype(np.int64),
                          np.arange(N_NODES, dtype=np.int64)])
    srow = row_of[src]
    dkey = pkey_of[dst]

    # window class: 0 = forced A (row < WIN_B_BASE), 1 = flex, 2 = forced B
    cls = np.where(srow < WIN_B_BASE, 0, np.where(srow < WIN_A_LEN, 1, 2))

    # sort edges by (dst node, class) so each node's edges are A,flex,B
    e_order = np.lexsort((cls, dkey))
    srow_s = srow[e_order]
    dkey_s = dkey[e_order]
    cls_s = cls[e_order]

    # within-node edge position j
    counts = np.bincount(dkey_s, minlength=N_CORES * PER_CORE)
    starts = np.zeros(N_CORES * PER_CORE, np.int64)
    np.cumsum(counts[:-1], out=starts[1:])
    j_in_node = np.arange(dkey_s.shape[0]) - starts[dkey_s]

    aF = np.bincount(dkey_s[cls_s == 0], minlength=N_CORES * PER_CORE)
    fl = np.bincount(dkey_s[cls_s == 1], minlength=N_CORES * PER_CORE)
    degp = counts                                     # == deg per processing pos

    # block grid dims shared across cores: per block b, tA_b/tB_b
    def to_cbp(arr):
        out = np.zeros((N_CORES, NBLK * BLK), np.int64)
        out[:, :PER_CORE] = arr.reshape(N_CORES, PER_CORE)
        return out.reshape(N_CORES, NBLK, BLK)

    aF_c = to_cbp(aF)
    fl_c = to_cbp(fl)
    deg_c = to_cbp(degp)
    tA = np.zeros(NBLK, np.int64)
    tB = np.zeros(NBLK, np.int64)
    for b in range(NBLK):
        a_ = aF_c[:, b, :].ravel()
        f_ = fl_c[:, b, :].ravel()
        d_ = deg_c[:, b, :].ravel()
        lo = int(a_.max())
        best = None
        for ta in range(lo, lo + 48):
            nA = np.minimum(a_ + f_, ta)
            tb = int((d_ - nA).max())
            if best is None or ta + tb < best[0] + best[1]:
                best = (ta, tb)
            if tb == 0:
                break
        tA[b], tB[b] = best

    # per-edge grid assignment
    tA_of_edge = tA[(dkey_s % PER_CORE) // BLK]
    aF_e = aF[dkey_s]
    fl_e = fl[dkey_s]
    nA_e = np.minimum(aF_e + fl_e, tA_of_edge)        # per-node nA, per edge
    in_A = j_in_node < nA_e
    colA = j_in_node                                  # valid where in_A
    colB = j_in_node - nA_e                           # valid where ~in_A
    idxA_val = srow_s                                 # window A index = row
    idxB_val = srow_s - WIN_B_BASE

    # group layout (shared): groups of GROUP blocks
    groups = []
    for g0 in range(0, NBLK, GROUP):
        blocks = list(range(g0, min(g0 + GROUP, NBLK)))
        sumA = int(tA[blocks].sum())
        sumB = int(tB[blocks].sum())
        colA0 = np.zeros(len(blocks), np.int64)
        colB0 = np.zeros(len(blocks), np.int64)
        np.cumsum(tA[blocks][:-1], out=colA0[1:])
        np.cumsum(tB[blocks][:-1], out=colB0[1:])
        groups.append(dict(blocks=blocks, sumA=sumA, sumB=sumB,
                           colA0=colA0, colB0=colB0))

    # per-edge flat stream position within its core's concatenated streams
    blk_of_edge = (dkey_s % PER_CORE) // BLK
    p_of_edge = (dkey_s % PER_CORE) % BLK
    core_of_edge = dkey_s // PER_CORE
    # offsets of each block's grid within the concatenated per-core stream
    offA_blk = np.zeros(NBLK, np.int64)   # in columns, within full A stream
    offB_blk = np.zeros(NBLK, np.int64)
    offA_grp = []
    offB_grp = []
    accA = accB = 0
    for g in groups:
        offA_grp.append(accA)
        offB_grp.append(accB)
        for i, b in enumerate(g["blocks"]):
            offA_blk[b] = accA + g["colA0"][i]
            offB_blk[b] = accB + g["colB0"][i]
        accA += g["sumA"]
        accB += g["sumB"]
    lenA, lenB = accA, accB               # total columns per stream

    # build per-core unwrapped idx streams
    idxA_streams = np.zeros((N_CORES, lenA * BLK), np.int64)
    idxB_streams = np.zeros((N_CORES, lenB * BLK), np.int64)
    eA = in_A
    posA = (offA_blk[blk_of_edge[eA]] + colA[eA]) * BLK + p_of_edge[eA]
    idxA_streams[core_of_edge[eA], posA] = idxA_val[eA]
    eB = ~in_A
    posB = (offB_blk[blk_of_edge[eB]] + colB[eB]) * BLK + p_of_edge[eB]
    idxB_streams[core_of_edge[eB], posB] = idxB_val[eB]

    # per-core deg in block layout [128, NBLK] (fake slots get deg=1)
    deg_blk = np.ones((N_CORES, NBLK * BLK), np.float32)
    deg_blk[:, :PER_CORE] = degp.reshape(N_CORES, PER_CORE).astype(np.float32)
    deg_blk = deg_blk.reshape(N_CORES, NBLK, BLK).transpose(0, 2, 1)  # [c,128,NBLK]

    meta = dict(tA=tA, tB=tB, groups=groups, lenA=lenA, lenB=lenB,
                offA_grp=offA_grp, offB_grp=offB_grp)
    percore = dict(order=order, core_of=core_of, pos_of=pos_of,
                   idxA=idxA_streams, idxB=idxB_streams, deg_blk=deg_blk)
    return meta, percore


def _build_nc(meta):
    nc = bacc.Bacc(None, target_bir_lowering=False, num_swdge_queues=4)
    lenA, lenB = meta["lenA"], meta["lenB"]
    tA, tB, groups = meta["tA"], meta["tB"], meta["groups"]

    # xT staged tiled: [t, k, p, n] = x[node (t*128+n), k*128+p]
    xT_d = nc.dram_tensor("xT", [NBLK, 2, 128, BLK], F32, kind="ExternalInput")
    w1_d = nc.dram_tensor("w1", [IN_DIM, HID_DIM], F32, kind="ExternalInput")
    w2_d = nc.dram_tensor("w2", [HID_DIM, OUT_DIM], F32, kind="ExternalInput")
    b1_d = nc.dram_tensor("b1b", [BLK, HID_DIM], F32, kind="ExternalInput")
    b2_d = nc.dram_tensor("b2b", [BLK, OUT_DIM], F32, kind="ExternalInput")
    ident_d = nc.dram_tensor("ident", [BLK, BLK], F32, kind="ExternalInput")
    deg_d = nc.dram_tensor("deg", [BLK, NBLK], F32, kind="ExternalInput")
    idxA_d = nc.dram_tensor("idxA", [128, lenA * 8], I16, kind="ExternalInput")
    idxB_d = nc.dram_tensor("idxB", [128, lenB * 8], I16, kind="ExternalInput")
    out_d = nc.dram_tensor("out", [PER_CORE, OUT_DIM], F32, kind="ExternalOutput")

    with tile.TileContext(nc) as tc:
        with (
            tc.tile_pool(name="dram", bufs=1, space="DRAM") as dram,
            tc.tile_pool(name="const", bufs=1) as cpool,
            tc.tile_pool(name="xpool", bufs=3) as xpool,
            tc.tile_pool(name="gpool", bufs=3) as gpool,
            tc.tile_pool(name="zpool", bufs=3) as zpool,
            tc.tile_pool(name="ps", bufs=2, space="PSUM") as ps,
            tc.tile_pool(name="ps2", bufs=2, space="PSUM") as ps2,
        ):
            t1_slice = dram.tile([SLICE, HID_DIM], F32)
            t1_full = dram.tile([TBL, HID_DIM], F32, addr_space="Shared")
            t2_slice = dram.tile([SLICE, OUT_DIM], F32)
            t2_full = dram.tile([TBL, OUT_DIM], F32, addr_space="Shared")

            # ---- constants ----
            # w1[p, k, :] = W1[k*128+p, :] so w1[:, k, :] is the k-th K-tile
            w1 = cpool.tile([128, 2, HID_DIM], F32)
            nc.sync.dma_start(w1[:], w1_d.rearrange("(a b) c -> b a c", a=2))
            w2 = cpool.tile([HID_DIM, OUT_DIM], F32)
            nc.sync.dma_start(w2[:], w2_d[:])
            b1b = cpool.tile([BLK, HID_DIM], F32)
            nc.sync.dma_start(b1b[:], b1_d[:])
            b2b = cpool.tile([BLK, OUT_DIM], F32)
            nc.sync.dma_start(b2b[:], b2_d[:])
            ident = cpool.tile([BLK, BLK], F32)
            nc.sync.dma_start(ident[:], ident_d[:])
            idxA = cpool.tile([128, lenA * 8], I16)
            nc.sync.dma_start(idxA[:], idxA_d[:])
            idxB = cpool.tile([128, lenB * 8], I16)
            nc.sync.dma_start(idxB[:], idxB_d[:])
            deg = cpool.tile([BLK, NBLK], F32)
            nc.sync.dma_start(deg[:], deg_d[:])
            dinv = cpool.tile([BLK, NBLK], F32)
            nc.scalar.sqrt(dinv[:], deg[:])
            nc.vector.reciprocal(dinv[:], dinv[:])
            zrow = cpool.tile([2, HID_DIM], F32)
            nc.vector.memset(zrow[:], 0.0)
            nc.sync.dma_start(t1_slice[0:2, :], zrow[:])
            nc.sync.dma_start(t2_slice[0:2, :], zrow[:, 0:OUT_DIM])

            # ---- phase 1: T1 slice = dinv * (x @ W1), in table order ----
            for t in range(NBLK):
                xt0 = xpool.tile([128, BLK], F32, tag="xt0", bufs=3)
                nc.sync.dma_start(xt0[:], xT_d[t, 0])
                xt1 = xpool.tile([128, BLK], F32, tag="xt1", bufs=3)
                nc.sync.dma_start(xt1[:], xT_d[t, 1])
                pst = ps.tile([BLK, HID_DIM], F32, tag="mm1")
                nc.tensor.matmul(pst[:], xt0[:], w1[:, 0, :], start=True, stop=False)
                nc.tensor.matmul(pst[:], xt1[:], w1[:, 1, :], start=False, stop=True)
                xw = zpool.tile([BLK, HID_DIM], F32, tag="xw")
                nc.scalar.mul(xw[:], pst[:], dinv[:, t:t + 1])
                hi = min((t + 1) * BLK, PER_CORE)
                nc.sync.dma_start(t1_slice[2 + t * BLK:2 + hi, :],
                                  xw[0:hi - t * BLK, :])

            # ---- phase 2: AllGather layer-1 table ----
            nc.gpsimd.collective_compute(
                "AllGather", mybir.AluOpType.bypass,
                replica_groups=[list(range(N_CORES))],
                ins=[t1_slice.opt()], outs=[t1_full.opt()],
            )

            # ---- phases 3+5 share this per-layer aggregation structure ----
            # Hoist num_idxs registers: one per distinct value, written once,
            # so gathers only read them (per-call to_reg MOVEs would WAR-
            # serialize the gather pipeline on register reuse).
            reg_cache = {}

            def nreg(v):
                if v not in reg_cache:
                    reg_cache[v] = nc.gpsimd.to_reg(v)
                return reg_cache[v]

            def aggregate(layer, tbl_full, fdim, idx_sb_A, idx_sb_B, consume):
                for gi, g in enumerate(groups):
                    sumA, sumB = g["sumA"], g["sumB"]
                    gt = gpool.tile([128, (sumA + sumB), fdim], F32, tag="G")
                    oA = meta["offA_grp"][gi] * 8
                    oB = meta["offB_grp"][gi] * 8
                    nc.gpsimd.dma_gather(
                        gt[:, 0:sumA, :], tbl_full[0:WIN_A_LEN, :],
                        idx_sb_A[:, oA:oA + sumA * 8],
                        sumA * BLK, nreg(sumA * BLK), fdim, single_packet=False,
                        queue_num=(2 * gi) % 4)
                    nc.gpsimd.dma_gather(
                        gt[:, sumA:sumA + sumB, :],
                        tbl_full[WIN_B_BASE:WIN_B_BASE + WIN_B_LEN, :],
                        idx_sb_B[:, oB:oB + sumB * 8],
                        sumB * BLK, nreg(sumB * BLK), fdim, single_packet=False,
                        queue_num=(2 * gi + 1) % 4)
                    import os
                    if os.environ.get("GCN_NOREDUCE") and layer == 1:
                        for i, b in enumerate(g["blocks"]):
                            z = zpool.tile([BLK, fdim], F32, tag=f"z{layer}")
                            nc.vector.tensor_copy(z[:], gt[:, 0, :])
                            consume(b, z)
                        continue
                    for i, b in enumerate(g["blocks"]):
                        # tree-reduce block b's A-range and B-range
                        cA0, nA = int(g["colA0"][i]), int(tA[b])
                        cB0, nB = sumA + int(g["colB0"][i]), int(tB[b])

                        def reduce_range(c0, n):
                            while n > 1:
                                h = n // 2
                                nc.vector.tensor_add(
                                    gt[:, c0:c0 + h, :],
                                    gt[:, c0:c0 + h, :],
                                    gt[:, c0 + n - h:c0 + n, :])
                                n -= h
                            return c0

                        z = zpool.tile([BLK, fdim], F32, tag=f"z{layer}")
                        if nA > 0 and nB > 0:
                            ra = reduce_range(cA0, nA)
                            rb = reduce_range(cB0, nB)
                            nc.vector.tensor_add(z[:], gt[:, ra, :], gt[:, rb, :])
                        elif nA > 0:
                            ra = reduce_range(cA0, nA)
                            nc.vector.tensor_copy(z[:], gt[:, ra, :])
                        else:
                            rb = reduce_range(cB0, nB)
                            nc.vector.tensor_copy(z[:], gt[:, rb, :])
                        consume(b, z)

            # ---- phase 3: aggregate layer 1, produce T2 slice ----
            def consume1(b, z):
                # z = sum of dinv*xw over in-edges; h'' = dinv*tanh(dinv*z+b1)
                nc.scalar.mul(z[:], z[:], dinv[:, b:b + 1])
                nc.vector.tensor_add(z[:], z[:], b1b[:])
                nc.scalar.activation(z[:], z[:], mybir.ActivationFunctionType.Tanh)
                nc.scalar.mul(z[:], z[:], dinv[:, b:b + 1])
                # t2 rows = h'' @ W2 (transpose h'' via PE, then matmul)
                pst = ps.tile([BLK, BLK], F32, tag="tr")
                nc.tensor.transpose(pst[:], z[:], ident[:])
                zT = zpool.tile([BLK, BLK], F32, tag="zT")
                nc.vector.tensor_copy(zT[:], pst[:])
                po = ps2.tile([BLK, OUT_DIM], F32, tag="mm2")
                nc.tensor.matmul(po[:], zT[:], w2[:], start=True, stop=True)
                t2row = zpool.tile([BLK, OUT_DIM], F32, tag="t2row")
                nc.vector.tensor_copy(t2row[:], po[:])
                hi = min((b + 1) * BLK, PER_CORE)
                nc.sync.dma_start(t2_slice[2 + b * BLK:2 + hi, :],
                                  t2row[0:hi - b * BLK, :])

            aggregate(1, t1_full, HID_DIM, idxA, idxB, consume1)

            # ---- phase 4: AllGather layer-2 table ----
            nc.gpsimd.collective_compute(
                "AllGather", mybir.AluOpType.bypass,
                replica_groups=[list(range(N_CORES))],
                ins=[t2_slice.opt()], outs=[t2_full.opt()],
            )

            # ---- phase 5: aggregate layer 2, write output ----
            def consume2(b, z):
                nc.scalar.mul(z[:], z[:], dinv[:, b:b + 1])
                nc.vector.tensor_add(z[:], z[:], b2b[:])
                hi = min((b + 1) * BLK, PER_CORE)
                nc.sync.dma_start(out_d[b * BLK:hi, :], z[0:hi - b * BLK, :])

            aggregate(2, t2_full, OUT_DIM, idxA, idxB, consume2)

    nc.compile()
    return nc


def _make_in_maps(x, W1, b1, W2, b2, meta, percore):
    order = percore["order"]
    core_nodes = [order[c::N_CORES] for c in range(N_CORES)]  # table order
    ident = np.eye(BLK, dtype=np.float32)
    b1b = np.broadcast_to(b1, (BLK, HID_DIM)).copy()
    b2b = np.broadcast_to(b2, (BLK, OUT_DIM)).copy()
    in_maps = []
    for c in range(N_CORES):
        xc = np.zeros((NBLK * BLK, IN_DIM), np.float32)
        xc[:PER_CORE] = x[core_nodes[c]]
        # [t, k, p, n] = x[node t*128+n, k*128+p]
        xT_tiled = np.ascontiguousarray(
            xc.reshape(NBLK, BLK, 2, 128).transpose(0, 2, 3, 1))
        in_maps.append({
            "xT": xT_tiled,
            "w1": W1, "w2": W2, "b1b": b1b, "b2b": b2b, "ident": ident,
            "deg": percore["deg_blk"][c],
            "idxA": _wrap_idxs(percore["idxA"][c]),
            "idxB": _wrap_idxs(percore["idxB"][c]),
        })
    return in_maps, core_nodes


_CACHE = {}


def _get_compiled(edge_index):
    key = hash(edge_index.tobytes())
    if key not in _CACHE:
        meta, percore = _preprocess(np.asarray(edge_index))
        nc = _build_nc(meta)
        _CACHE[key] = (nc, meta, percore)
    return _CACHE[key]


def run(x, edge_index, W1, b1, W2, b2, trace=False):
    x = np.asarray(x, dtype=np.float32)
    W1 = np.asarray(W1, dtype=np.float32)
    W2 = np.asarray(W2, dtype=np.float32)
    b1 = np.asarray(b1, dtype=np.float32)
    b2 = np.asarray(b2, dtype=np.float32)
    nc, meta, percore = _get_compiled(np.asarray(edge_index))
    in_maps, core_nodes = _make_in_maps(x, W1, b1, W2, b2, meta, percore)
    sim = MultiCoreSim(nc, N_CORES)
    try:
        res = sim.run_on_hw_raw(in_maps=in_maps, trace=trace)
    except Exception:
        res = sim.run_on_hw_raw(in_maps=in_maps, trace=trace)  # retry transient
    out = np.zeros((N_NODES, OUT_DIM), np.float32)
    for c in range(N_CORES):
        out[core_nodes[c]] = res.results[c]["out"]
    return out, res


def kernel(**inputs) -> np.ndarray:
    out, _ = run(**inputs)
    return out


# revision 21
# speedup vs baseline: 1.0381x; 1.0381x over previous
"""Two-layer GCN (PyG GCNConv x2 with tanh) on 8 Trainium2 NeuronCores.

Strategy (graph/data parallel, per sharding hint):
  - Nodes are dealt round-robin by degree rank across the 8 cores (balances
    edge counts), then degree-sorted within each core into blocks of 128.
  - The "table" (per-layer per-node feature rows) lives in DRAM in permuted
    node order: 8 chunks of 6252 rows (2 leading zero rows per chunk, used
    as gather padding targets), AllGathered across cores after each local
    transform.
  - Normalization trick: out[d] = dinv[d] * sum_{e:dst=d} (dinv[s]*xw[s]),
    so the table rows are pre-scaled by dinv and the aggregation is a pure
    unweighted segment-sum of gathered rows.
  - Aggregation: per 128-node block, dma_gather pulls all incident edges'
    table rows into SBUF slots [partition=node, column=edge j], and the DVE
    tree-reduces columns. Gather indices are int16, so each gather call uses
    one of two overlapping table windows (rows [0,32768) / [18756,50016))
    with per-edge window assignment balanced at preprocessing time.
  - Layer 1 transforms then aggregates (128-dim rows); layer 2 transforms
    (h @ W2) then aggregates 64-dim rows.
"""
import sys

if "/opt/trn_rl_repo" not in sys.path:
    sys.path.insert(0, "/opt/trn_rl_repo")

import numpy as np

import concourse.bacc as bacc
import concourse.mybir as mybir
import concourse.tile as tile
from concourse.bass_interp import MultiCoreSim

# -------- problem constants (hardcoded; kernel.py must be self-contained) ----
N_NODES = 50000
IN_DIM, HID_DIM, OUT_DIM = 256, 128, 64
N_CORES = 8
PER_CORE = N_NODES // N_CORES          # 6250
SLICE = PER_CORE + 2                   # 6252 (2 zero rows per core chunk)
TBL = SLICE * N_CORES                  # 50016
WIN_A_LEN = 32768                      # window A: rows [0, 32768)
WIN_B_BASE = 3 * SLICE                 # 18756 == a zero row (core 3, row 0)
WIN_B_LEN = TBL - WIN_B_BASE           # 31260 (max idx 31259 < 32768)
BLK = 128
NBLK = (PER_CORE + BLK - 1) // BLK     # 49 (last block has 106 real nodes)
GROUP = 1                              # blocks per gather call group
F32 = mybir.dt.float32
I16 = mybir.dt.int16


def _wrap_idxs(idx_flat: np.ndarray) -> np.ndarray:
    """Unwrapped idx stream [n] -> [128, n//16] int16 (16-partition wrap,
    replicated 8x for the 8 Q7 cores)."""
    n = idx_flat.shape[0]
    assert n % 16 == 0
    base = idx_flat.reshape(n // 16, 16).T.astype(np.int16)
    return np.tile(base, (8, 1))


def _preprocess(edge_index: np.ndarray):
    """Host-side integer/index preprocessing. Returns shared structure
    (block grid dims, group layout) and per-core staged arrays."""
    deg = np.bincount(edge_index[1].astype(np.int64), minlength=N_NODES) + 1
    deg = deg.astype(np.int64)  # in-degree including self loop

    # node -> (core, pos, table row): round-robin deal in degree-desc order
    order = np.argsort(-deg, kind="stable")          # rank -> node
    rank = np.empty(N_NODES, np.int64)
    rank[order] = np.arange(N_NODES)
    core_of = rank % N_CORES
    pos_of = rank // N_CORES                          # 0..6249 within core
    row_of = core_of * SLICE + 2 + pos_of             # table row of node
    # processing key: core*PER_CORE + pos (0..49999)
    pkey_of = core_of * PER_CORE + pos_of

    # edges incl self loops, as (src_row, dst_pkey)
    src = np.concatenate([edge_index[0].astype(np.int64),
                          np.arange(N_NODES, dtype=np.int64)])
    dst = np.concatenate([edge_index[1].astype(np.int64),
                          np.arange(N_NODES, dtype=np.int64)])
    srow = row_of[src]
    dkey = pkey_of[dst]

    # window class: 0 = forced A (row < WIN_B_BASE), 1 = flex, 2 = forced B
    cls = np.where(srow < WIN_B_BASE, 0, np.where(srow < WIN_A_LEN, 1, 2))

    # sort edges by (dst node, class) so each node's edges are A,flex,B
    e_order = np.lexsort((cls, dkey))
    srow_s = srow[e_order]
    dkey_s = dkey[e_order]
    cls_s = cls[e_order]

    # within-node edge position j
    counts = np.bincount(dkey_s, minlength=N_CORES * PER_CORE)
    starts = np.zeros(N_CORES * PER_CORE, np.int64)
    np.cumsum(counts[:-1], out=starts[1:])
    j_in_node = np.arange(dkey_s.shape[0]) - starts[dkey_s]

    aF = np.bincount(dkey_s[cls_s == 0], minlength=N_CORES * PER_CORE)
    fl = np.bincount(dkey_s[cls_s == 1], minlength=N_CORES * PER_CORE)
    degp = counts                                     # == deg per processing pos

    # block grid dims shared across cores: per block b, tA_b/tB_b
    def to_cbp(arr):
        out = np.zeros((N_CORES, NBLK * BLK), np.int64)
        out[:, :PER_CORE] = arr.reshape(N_CORES, PER_CORE)
        return out.reshape(N_CORES, NBLK, BLK)

    aF_c = to_cbp(aF)
    fl_c = to_cbp(fl)
    deg_c = to_cbp(degp)
    tA = np.zeros(NBLK, np.int64)
    tB = np.zeros(NBLK, np.int64)
    for b in range(NBLK):
        a_ = aF_c[:, b, :].ravel()
        f_ = fl_c[:, b, :].ravel()
        d_ = deg_c[:, b, :].ravel()
        lo = int(a_.max())
        best = None
        for ta in range(lo, lo + 48):
            nA = np.minimum(a_ + f_, ta)
            tb = int((d_ - nA).max())
            if best is None or ta + tb < best[0] + best[1]:
                best = (ta, tb)
            if tb == 0:
                break
        tA[b], tB[b] = best

    # per-edge grid assignment
    tA_of_edge = tA[(dkey_s % PER_CORE) // BLK]
    aF_e = aF[dkey_s]
    fl_e = fl[dkey_s]
    nA_e = np.minimum(aF_e + fl_e, tA_of_edge)        # per-node nA, per edge
    in_A = j_in_node < nA_e
    colA = j_in_node                                  # valid where in_A
    colB = j_in_node - nA_e                           # valid where ~in_A
    idxA_val = srow_s                                 # window A index = row
    idxB_val = srow_s - WIN_B_BASE

    # group layout (shared): groups of GROUP blocks
    groups = []
    for g0 in range(0, NBLK, GROUP):
        blocks = list(range(g0, min(g0 + GROUP, NBLK)))
        sumA = int(tA[blocks].sum())
        sumB = int(tB[blocks].sum())
        colA0 = np.zeros(len(blocks), np.int64)
        colB0 = np.zeros(len(blocks), np.int64)
        np.cumsum(tA[blocks][:-1], out=colA0[1:])
        np.cumsum(tB[blocks][:-1], out=colB0[1:])
        groups.append(dict(blocks=blocks, sumA=sumA, sumB=sumB,
                           colA0=colA0, colB0=colB0))

    # per-edge flat stream position within its core's concatenated streams
    blk_of_edge = (dkey_s % PER_CORE) // BLK
    p_of_edge = (dkey_s % PER_CORE) % BLK
    core_of_edge = dkey_s // PER_CORE
    # offsets of each block's grid within the concatenated per-core stream
    offA_blk = np.zeros(NBLK, np.int64)   # in columns, within full A stream
    offB_blk = np.zeros(NBLK, np.int64)
    offA_grp = []
    offB_grp = []
    accA = accB = 0
    for g in groups:
        offA_grp.append(accA)
        offB_grp.append(accB)
        for i, b in enumerate(g["blocks"]):
            offA_blk[b] = accA + g["colA0"][i]
            offB_blk[b] = accB + g["colB0"][i]
        accA += g["sumA"]
        accB += g["sumB"]
    lenA, lenB = accA, accB               # total columns per stream

    # build per-core unwrapped idx streams
    idxA_streams = np.zeros((N_CORES, lenA * BLK), np.int64)
    idxB_streams = np.zeros((N_CORES, lenB * BLK), np.int64)
    eA = in_A
    posA = (offA_blk[blk_of_edge[eA]] + colA[eA]) * BLK + p_of_edge[eA]
    idxA_streams[core_of_edge[eA], posA] = idxA_val[eA]
    eB = ~in_A
    posB = (offB_blk[blk_of_edge[eB]] + colB[eB]) * BLK + p_of_edge[eB]
    idxB_streams[core_of_edge[eB], posB] = idxB_val[eB]

    # per-core deg in block layout [128, NBLK] (fake slots get deg=1)
    deg_blk = np.ones((N_CORES, NBLK * BLK), np.float32)
    deg_blk[:, :PER_CORE] = degp.reshape(N_CORES, PER_CORE).astype(np.float32)
    deg_blk = deg_blk.reshape(N_CORES, NBLK, BLK).transpose(0, 2, 1)  # [c,128,NBLK]

    meta = dict(tA=tA, tB=tB, groups=groups, lenA=lenA, lenB=lenB,
                offA_grp=offA_grp, offB_grp=offB_grp)
    percore = dict(order=order, core_of=core_of, pos_of=pos_of,
                   idxA=idxA_streams, idxB=idxB_streams, deg_blk=deg_blk)
    return meta, percore


def _build_nc(meta):
    nc = bacc.Bacc(None, target_bir_lowering=False, num_swdge_queues=4)
    lenA, lenB = meta["lenA"], meta["lenB"]
    tA, tB, groups = meta["tA"], meta["tB"], meta["groups"]

    # xT staged tiled: [t, k, p, n] = x[node (t*128+n), k*128+p]
    xT_d = nc.dram_tensor("xT", [NBLK, 2, 128, BLK], F32, kind="ExternalInput")
    w1_d = nc.dram_tensor("w1", [IN_DIM, HID_DIM], F32, kind="ExternalInput")
    w2_d = nc.dram_tensor("w2", [HID_DIM, OUT_DIM], F32, kind="ExternalInput")
    b1_d = nc.dram_tensor("b1b", [BLK, HID_DIM], F32, kind="ExternalInput")
    b2_d = nc.dram_tensor("b2b", [BLK, OUT_DIM], F32, kind="ExternalInput")
    ident_d = nc.dram_tensor("ident", [BLK, BLK], F32, kind="ExternalInput")
    deg_d = nc.dram_tensor("deg", [BLK, NBLK], F32, kind="ExternalInput")
    idxA_d = nc.dram_tensor("idxA", [128, lenA * 8], I16, kind="ExternalInput")
    idxB_d = nc.dram_tensor("idxB", [128, lenB * 8], I16, kind="ExternalInput")
    out_d = nc.dram_tensor("out", [PER_CORE, OUT_DIM], F32, kind="ExternalOutput")

    with tile.TileContext(nc) as tc:
        with (
            tc.tile_pool(name="dram", bufs=1, space="DRAM") as dram,
            tc.tile_pool(name="const", bufs=1) as cpool,
            tc.tile_pool(name="xpool", bufs=3) as xpool,
            tc.tile_pool(name="gpool", bufs=6) as gpool,
            tc.tile_pool(name="zpool", bufs=3) as zpool,
            tc.tile_pool(name="ps", bufs=2, space="PSUM") as ps,
            tc.tile_pool(name="ps2", bufs=2, space="PSUM") as ps2,
        ):
            t1_slice = dram.tile([SLICE, HID_DIM], F32)
            t1_full = dram.tile([TBL, HID_DIM], F32, addr_space="Shared")
            t2_slice = dram.tile([SLICE, OUT_DIM], F32)
            t2_full = dram.tile([TBL, OUT_DIM], F32, addr_space="Shared")

            # ---- constants ----
            # w1[p, k, :] = W1[k*128+p, :] so w1[:, k, :] is the k-th K-tile
            w1 = cpool.tile([128, 2, HID_DIM], F32)
            nc.sync.dma_start(w1[:], w1_d.rearrange("(a b) c -> b a c", a=2))
            w2 = cpool.tile([HID_DIM, OUT_DIM], F32)
            nc.sync.dma_start(w2[:], w2_d[:])
            b1b = cpool.tile([BLK, HID_DIM], F32)
            nc.sync.dma_start(b1b[:], b1_d[:])
            b2b = cpool.tile([BLK, OUT_DIM], F32)
            nc.sync.dma_start(b2b[:], b2_d[:])
            ident = cpool.tile([BLK, BLK], F32)
            nc.sync.dma_start(ident[:], ident_d[:])
            idxA = cpool.tile([128, lenA * 8], I16)
            nc.sync.dma_start(idxA[:], idxA_d[:])
            idxB = cpool.tile([128, lenB * 8], I16)
            nc.sync.dma_start(idxB[:], idxB_d[:])
            deg = cpool.tile([BLK, NBLK], F32)
            nc.sync.dma_start(deg[:], deg_d[:])
            dinv = cpool.tile([BLK, NBLK], F32)
            nc.scalar.sqrt(dinv[:], deg[:])
            nc.vector.reciprocal(dinv[:], dinv[:])
            zrow = cpool.tile([2, HID_DIM], F32)
            nc.vector.memset(zrow[:], 0.0)
            nc.sync.dma_start(t1_slice[0:2, :], zrow[:])
            nc.sync.dma_start(t2_slice[0:2, :], zrow[:, 0:OUT_DIM])

            # ---- phase 1: T1 slice = dinv * (x @ W1), in table order ----
            for t in range(NBLK):
                xt0 = xpool.tile([128, BLK], F32, tag="xt0", bufs=3)
                nc.sync.dma_start(xt0[:], xT_d[t, 0])
                xt1 = xpool.tile([128, BLK], F32, tag="xt1", bufs=3)
                nc.sync.dma_start(xt1[:], xT_d[t, 1])
                pst = ps.tile([BLK, HID_DIM], F32, tag="mm1")
                nc.tensor.matmul(pst[:], xt0[:], w1[:, 0, :], start=True, stop=False)
                nc.tensor.matmul(pst[:], xt1[:], w1[:, 1, :], start=False, stop=True)
                xw = zpool.tile([BLK, HID_DIM], F32, tag="xw")
                nc.scalar.mul(xw[:], pst[:], dinv[:, t:t + 1])
                hi = min((t + 1) * BLK, PER_CORE)
                nc.sync.dma_start(t1_slice[2 + t * BLK:2 + hi, :],
                                  xw[0:hi - t * BLK, :])

            # ---- phase 2: AllGather layer-1 table ----
            nc.gpsimd.collective_compute(
                "AllGather", mybir.AluOpType.bypass,
                replica_groups=[list(range(N_CORES))],
                ins=[t1_slice.opt()], outs=[t1_full.opt()],
            )

            # ---- phases 3+5 share this per-layer aggregation structure ----
            # Hoist num_idxs registers: one per distinct value, written once,
            # so gathers only read them (per-call to_reg MOVEs would WAR-
            # serialize the gather pipeline on register reuse).
            reg_cache = {}

            def nreg(v):
                if v not in reg_cache:
                    reg_cache[v] = nc.gpsimd.to_reg(v)
                return reg_cache[v]

            def aggregate(layer, tbl_full, fdim, idx_sb_A, idx_sb_B, consume):
                for gi, g in enumerate(groups):
                    sumA, sumB = g["sumA"], g["sumB"]
                    gt = gpool.tile([128, (sumA + sumB), fdim], F32, tag="G")
                    oA = meta["offA_grp"][gi] * 8
                    oB = meta["offB_grp"][gi] * 8
                    nc.gpsimd.dma_gather(
                        gt[:, 0:sumA, :], tbl_full[0:WIN_A_LEN, :],
                        idx_sb_A[:, oA:oA + sumA * 8],
                        sumA * BLK, nreg(sumA * BLK), fdim, single_packet=False,
                        queue_num=(2 * gi) % 4)
                    nc.gpsimd.dma_gather(
                        gt[:, sumA:sumA + sumB, :],
                        tbl_full[WIN_B_BASE:WIN_B_BASE + WIN_B_LEN, :],
                        idx_sb_B[:, oB:oB + sumB * 8],
                        sumB * BLK, nreg(sumB * BLK), fdim, single_packet=False,
                        queue_num=(2 * gi + 1) % 4)
                    for i, b in enumerate(g["blocks"]):
                        # tree-reduce block b's A-range and B-range
                        cA0, nA = int(g["colA0"][i]), int(tA[b])
                        cB0, nB = sumA + int(g["colB0"][i]), int(tB[b])

                        def reduce_range(c0, n):
                            while n > 1:
                                h = n // 2
                                nc.vector.tensor_add(
                                    gt[:, c0:c0 + h, :],
                                    gt[:, c0:c0 + h, :],
                                    gt[:, c0 + n - h:c0 + n, :])
                                n -= h
                            return c0

                        z = zpool.tile([BLK, fdim], F32, tag=f"z{layer}")
                        if nA > 0 and nB > 0:
                            ra = reduce_range(cA0, nA)
                            rb = reduce_range(cB0, nB)
                            nc.vector.tensor_add(z[:], gt[:, ra, :], gt[:, rb, :])
                        elif nA > 0:
                            ra = reduce_range(cA0, nA)
                            nc.vector.tensor_copy(z[:], gt[:, ra, :])
                        else:
                            rb = reduce_range(cB0, nB)
                            nc.vector.tensor_copy(z[:], gt[:, rb, :])
                        consume(b, z)

            # ---- phase 3: aggregate layer 1, produce T2 slice ----
            def consume1(b, z):
                # z = sum of dinv*xw over in-edges; h'' = dinv*tanh(dinv*z+b1)
                nc.scalar.mul(z[:], z[:], dinv[:, b:b + 1])
                nc.vector.tensor_add(z[:], z[:], b1b[:])
                nc.scalar.activation(z[:], z[:], mybir.ActivationFunctionType.Tanh)
                nc.scalar.mul(z[:], z[:], dinv[:, b:b + 1])
                # t2 rows = h'' @ W2 (transpose h'' via PE, then matmul)
                pst = ps.tile([BLK, BLK], F32, tag="tr")
                nc.tensor.transpose(pst[:], z[:], ident[:])
                zT = zpool.tile([BLK, BLK], F32, tag="zT")
                nc.vector.tensor_copy(zT[:], pst[:])
                po = ps2.tile([BLK, OUT_DIM], F32, tag="mm2")
                nc.tensor.matmul(po[:], zT[:], w2[:], start=True, stop=True)
                t2row = zpool.tile([BLK, OUT_DIM], F32, tag="t2row")
                nc.vector.tensor_copy(t2row[:], po[:])
                hi = min((b + 1) * BLK, PER_CORE)
                nc.sync.dma_start(t2_slice[2 + b * BLK:2 + hi, :],
                                  t2row[0:hi - b * BLK, :])

            aggregate(1, t1_full, HID_DIM, idxA, idxB, consume1)

            # ---- phase 4: AllGather layer-2 table ----
            nc.gpsimd.collective_compute(
                "AllGather", mybir.AluOpType.bypass,
                replica_groups=[list(range(N_CORES))],
                ins=[t2_slice.opt()], outs=[t2_full.opt()],
            )

            # ---- phase 5: aggregate layer 2, write output ----
            def consume2(b, z):
                nc.scalar.mul(z[:], z[:], dinv[:, b:b + 1])
                nc.vector.tensor_add(z[:], z[:], b2b[:])
                hi = min((b + 1) * BLK, PER_CORE)
                nc.sync.dma_start(out_d[b * BLK:hi, :], z[0:hi - b * BLK, :])

            aggregate(2, t2_full, OUT_DIM, idxA, idxB, consume2)

    nc.compile()
    return nc


def _make_in_maps(x, W1, b1, W2, b2, meta, percore):
    order = percore["order"]
    core_nodes = [order[c::N_CORES] for c in range(N_CORES)]  # table order
    ident = np.eye(BLK, dtype=np.float32)
    b1b = np.broadcast_to(b1, (BLK, HID_DIM)).copy()
    b2b = np.broadcast_to(b2, (BLK, OUT_DIM)).copy()
    in_maps = []
    for c in range(N_CORES):
        xc = np.zeros((NBLK * BLK, IN_DIM), np.float32)
        xc[:PER_CORE] = x[core_nodes[c]]
        # [t, k, p, n] = x[node t*128+n, k*128+p]
        xT_tiled = np.ascontiguousarray(
            xc.reshape(NBLK, BLK, 2, 128).transpose(0, 2, 3, 1))
        in_maps.append({
            "xT": xT_tiled,
            "w1": W1, "w2": W2, "b1b": b1b, "b2b": b2b, "ident": ident,
            "deg": percore["deg_blk"][c],
            "idxA": _wrap_idxs(percore["idxA"][c]),
            "idxB": _wrap_idxs(percore["idxB"][c]),
        })
    return in_maps, core_nodes


_CACHE = {}


def _get_compiled(edge_index):
    key = hash(edge_index.tobytes())
    if key not in _CACHE:
        meta, percore = _preprocess(np.asarray(edge_index))
        nc = _build_nc(meta)
        _CACHE[key] = (nc, meta, percore)
    return _CACHE[key]


def run(x, edge_index, W1, b1, W2, b2, trace=False):
    x = np.asarray(x, dtype=np.float32)
    W1 = np.asarray(W1, dtype=np.float32)
    W2 = np.asarray(W2, dtype=np.float32)
    b1 = np.asarray(b1, dtype=np.float32)
    b2 = np.asarray(b2, dtype=np.float32)
    nc, meta, percore = _get_compiled(np.asarray(edge_index))
    in_maps, core_nodes = _make_in_maps(x, W1, b1, W2, b2, meta, percore)
    sim = MultiCoreSim(nc, N_CORES)
    try:
        res = sim.run_on_hw_raw(in_maps=in_maps, trace=trace)
    except Exception:
        res = sim.run_on_hw_raw(in_maps=in_maps, trace=trace)  # retry transient
    out = np.zeros((N_NODES, OUT_DIM), np.float32)
    for c in range(N_CORES):
        out[core_nodes[c]] = res.results[c]["out"]
    return out, res


def kernel(**inputs) -> np.ndarray:
    out, _ = run(**inputs)
    return out
